# revision 3
# baseline (speedup 1.0000x reference)
"""Multi-head attention (B=4, N=2048, D=1024, H=16) on 8 trn2 cores, v2.

Sharding: core c -> (batch b = c//2, head-group g = c%2), 8 heads/core.
Host pre-transposes x (-> [D, N]) and casts x/weights to bf16, so the
device runs a pure bf16 matmul pipeline with no PE transposes and no
weight-rounding passes.  Host sums the per-pair projection partials.

Device dataflow per core:
  xp[dt]    : x^T resident in SBUF (DMA, bf16)
  v_aug[kt] : [128 keys, 8*(64+1)] v columns + ones column per head
  qk stage  : per head-pair, q'/k' [128, N] bf16 SBUF-resident; even head
              reads partitions 0..63, odd head 64..127 (PE row tiling —
              fine in bf16; the fp32r base-64 crash does not apply)
  attention : per (head, 512-q window): 8 strips of 2 key tiles,
              software-pipelined one strip ahead of the ACT exp;
              QK -> psum [128,1024] (double-buffered), exp (ACT, 1024
              wide) -> p bf16, PV accumulates o_ps [65,512] (row 64 =
              softmax denom via ones column); epilogue: denom -> DRAM
              -> stride-0 bcast, reciprocal, mul -> ostack bf16
  proj      : per pair, y_part = ostack_p^T @ wp rows -> bf16 partial,
              DMA'd out; host adds the 4 pair partials + b_proj.

A deficit ledger interleaves the dense work (v_aug tiles, next pair's
qk sections, previous pair's proj) into the attention strip stream as
PE filler so the tensor engine never idles waiting on the ACT exp.
"""
import sys

sys.path.insert(0, "/opt/trn_rl_repo")

from collections import deque

import numpy as np

import concourse.bass as bass
import concourse.mybir as mybir
import concourse.tile as tile
from concourse import bacc
from concourse.bass_utils import run_bass_kernel_spmd

F32 = mybir.dt.float32
BF16 = mybir.dt.bfloat16
AF = mybir.ActivationFunctionType

B = 4            # batch
N = 2048         # sequence length
D = 1024         # model dim
H = 16           # total heads
HL = 8           # heads per core
HD = 64          # head dim
SCALE = HD ** -0.5

import os

NKT = N // 128   # 16 key tiles
NDT = D // 128   # 8 d tiles
QWIN = int(os.environ.get("K2_QWIN", "512"))  # q window (best: 512)
NQW = N // QWIN
KPS = int(os.environ.get("K2_KPS", "2"))   # key tiles per strip
SPS_BUFS = int(os.environ.get("K2_SPSB", "2"))  # s_ps double-buffering
NSP = NKT // KPS # strips per (head, window)
FPH = NQW * (NSP + 1)  # feed slots per head

# PE cycles granted to the filler ledger per attention strip (the gap
# between ACT exp time and the strip's own QK+PV matmul time).
DEFICIT_C = 1050


def _build_nc(rep=1):
    nc = bacc.Bacc(None, target_bir_lowering=False)

    xt = nc.declare_dram_parameter("xt", [D, N], BF16, isOutput=False)
    wqk = nc.declare_dram_parameter("wqk", [D, D], BF16, isOutput=False)
    wv = nc.declare_dram_parameter("wv", [D, 512], BF16, isOutput=False)
    wp = nc.declare_dram_parameter("wp", [512, D], BF16, isOutput=False)
    yp = nc.declare_dram_parameter("yp", [4 * N, D], BF16, isOutput=True)

    with tile.TileContext(nc) as tc:
      with tc.tile_pool(name="dramp", bufs=1, space="DRAM") as dramp:
        dscr = [dramp.tile([NQW, QWIN], F32, tag=f"dscr{h}", name=f"dscr{h}")
                for h in range(HL)]
        for _rep in range(rep):
         with tc.tile_pool(name="wsb", bufs=1) as wsb, \
              tc.tile_pool(name="xr", bufs=1) as xrp, \
              tc.tile_pool(name="resv", bufs=1) as resv, \
              tc.tile_pool(name="qkst", bufs=4) as qkstp, \
              tc.tile_pool(name="ost", bufs=3) as ostp, \
              tc.tile_pool(name="pst", bufs=3) as pstp, \
              tc.tile_pool(name="epi", bufs=2) as epi, \
              tc.tile_pool(name="ypst", bufs=2) as ypstp, \
              tc.tile_pool(name="d_ps", bufs=2, space="PSUM") as d_psp, \
              tc.tile_pool(name="s_ps", bufs=SPS_BUFS, space="PSUM") as s_psp, \
              tc.tile_pool(name="o_ps", bufs=int(os.environ.get("K2_OPSB", "2")),
                           space="PSUM") as o_psp:

            # ---- resident loads (DMA only, bf16 straight from DRAM) ----
            xp = [xrp.tile([128, N], BF16, tag=f"xp{dt}", name=f"xp{dt}")
                  for dt in range(NDT)]
            wv_sb = [wsb.tile([128, 512], BF16, tag=f"wv{dt}", name=f"wv{dt}")
                     for dt in range(NDT)]
            wqk_sb = [wsb.tile([128, D], BF16, tag=f"wqk{dt}", name=f"wqk{dt}")
                      for dt in range(NDT)]
            wp_sb = [wsb.tile([128, D], BF16, tag=f"wp{p}", name=f"wp{p}")
                     for p in range(4)]
            for dt in range(NDT):
                nc.sync.dma_start(out=xp[dt],
                                  in_=xt[dt * 128:(dt + 1) * 128, :])
                nc.sync.dma_start(out=wv_sb[dt],
                                  in_=wv[dt * 128:(dt + 1) * 128, :])
            for dt in range(NDT):
                nc.sync.dma_start(out=wqk_sb[dt],
                                  in_=wqk[dt * 128:(dt + 1) * 128, :])
            for p in range(4):
                nc.sync.dma_start(out=wp_sb[p],
                                  in_=wp[p * 128:(p + 1) * 128, :])

            v_aug = [resv.tile([128, HL * (HD + 1)], BF16,
                               tag=f"va{kt}", name=f"va{kt}")
                     for kt in range(NKT)]

            # stage tiles for q'/k', SBUF-resident per pair
            qst = {}
            kst = {}

            # ---------------- dense work units ----------------
            def v_unit(kt):
                def fn():
                    ps = d_psp.tile([128, 512], F32, tag="dps")
                    for dt in range(NDT):
                        nc.tensor.matmul(
                            ps, xp[dt][:, kt * 128:(kt + 1) * 128], wv_sb[dt],
                            start=(dt == 0), stop=(dt == NDT - 1))
                    va3 = v_aug[kt].rearrange("p (h c) -> p h c", h=HL)
                    ps3 = ps.rearrange("p (h c) -> p h c", h=HL)
                    nc.vector.tensor_copy(va3[:, :, 0:HD], ps3)
                    nc.gpsimd.memset(va3[:, :, HD:HD + 1], 1.0)
                return fn

            def sec_unit(p, sec, rw):
                # sec 0 = q cols [p*128:(p+1)*128], sec 1 = k cols [512+...]
                def fn():
                    if rw == 0:
                        st = qkstp.tile([128, N], BF16, tag="qkst",
                                        name=f"st{p}_{sec}_{_rep}")
                        (qst if sec == 0 else kst)[p] = st
                    st = (qst if sec == 0 else kst)[p]
                    col0 = sec * 512 + p * 128
                    ps = d_psp.tile([128, 512], F32, tag="dps")
                    for dt in range(NDT):
                        nc.tensor.matmul(
                            ps, wqk_sb[dt][:, col0:col0 + 128],
                            xp[dt][:, rw * 512:(rw + 1) * 512],
                            start=(dt == 0), stop=(dt == NDT - 1))
                    nc.vector.tensor_copy(st[:, rw * 512:(rw + 1) * 512], ps)
                return fn

            ypst = {}

            def proj_unit(p, rt, half):
                def fn():
                    if half == 0:
                        ypst[p] = ypstp.tile([128, D], BF16, tag="ypst",
                                             name=f"yp{p}_{rt}_{_rep}")
                    ps = d_psp.tile([128, 512], F32, tag="dps")
                    nc.tensor.matmul(
                        ps, ostack[p][:, rt * 128:(rt + 1) * 128],
                        wp_sb[p][:, half * 512:(half + 1) * 512],
                        start=True, stop=True)
                    nc.vector.tensor_copy(
                        ypst[p][:, half * 512:(half + 1) * 512], ps)
                    if half == 1:
                        nc.sync.dma_start(
                            out=yp[p * N + rt * 128:p * N + (rt + 1) * 128, :],
                            in_=ypst[p])
                return fn

            # priority queue of (due_stream_idx, seq, cycles, fn), popped
            # earliest-due first, paced by a PE-cycle credit ledger
            import heapq
            queue = []
            seq_n = [0]

            def push(due, cyc, fn):
                heapq.heappush(queue, (due, seq_n[0], cyc, fn))
                seq_n[0] += 1

            ledger = {"credit": 0}

            def feed(stream):
                ledger["credit"] = min(ledger["credit"] + DEFICIT_C, 8192)
                while queue and (queue[0][0] <= stream
                                 or ledger["credit"] >= queue[0][2]):
                    _, _, cyc, fn = heapq.heappop(queue)
                    fn()
                    ledger["credit"] = max(ledger["credit"] - cyc, -20000)

            # ---------------- upfront phase ----------------
            # minimum preamble: first strip of h0/qw0 needs v_aug[0:KPS],
            # k' rows for key tiles 0..KPS-1 (k sec rw0) and q' window 0.
            for kt in range(KPS):
                v_unit(kt)()
            sec_unit(0, 1, 0)()
            sec_unit(0, 0, 0)()
            # remaining v tiles: strip sp consumes kt KPS*sp..KPS*sp+KPS-1
            for kt in range(KPS, NKT):
                push(max(0, kt // KPS - 1), 4096, v_unit(kt))
            # pair0's remaining sections, due just before their use
            for rw in range(1, 4):
                push(rw - 1, 4096, sec_unit(0, 1, rw))             # k rw
            for rw in range(1, 4):
                # q seq-range rw first used when its q window starts
                due = max(0, (NSP + 1) * (rw * 512 // QWIN) - 3)
                push(due, 4096, sec_unit(0, 0, rw))
            # later pairs' sections: spread across the prior pair's feeds
            for p in range(1, 4):
                base = (2 * p - 2) * FPH              # start of pair p-1
                for i, sec in enumerate((1, 0)):
                    for rw in range(4):
                        due = base + 4 + 4 * (4 * i + rw)
                        push(due, 4096, sec_unit(p, sec, rw))

            ostack = [None] * 4

            # ---------------- attention ----------------
            stream = 0
            for h in range(HL):
                p = h // 2
                half = slice((h % 2) * 64, (h % 2) * 64 + 64)
                k_t = kst[p]
                q_t = qst[p]
                if h % 2 == 0:
                    ostack[p] = ostp.tile([128, N], BF16, tag="ostk",
                                          name=f"os{p}_{_rep}")
                for qw in range(NQW):
                    qsl = slice(qw * QWIN, (qw + 1) * QWIN)
                    o_ps = o_psp.tile([65, QWIN], F32, tag="ops")
                    # software pipeline: QK(i)+exp(i) emitted a strip ahead
                    # of PV(i), so the in-order PE never waits on the ACT.
                    p_pipe = [None] * NSP
                    for sp in range(NSP + 1):
                        if sp < NSP:
                            s_ps = s_psp.tile([128, KPS * QWIN], F32,
                                              tag="sps")
                            for j in range(KPS):
                                kt = KPS * sp + j
                                nc.tensor.matmul(
                                    s_ps[:, j * QWIN:(j + 1) * QWIN],
                                    k_t[half, kt * 128:(kt + 1) * 128],
                                    q_t[half, qsl], start=True, stop=True)
                            p_sb = pstp.tile([128, KPS * QWIN], BF16,
                                             tag="psb")
                            nc.scalar.activation(p_sb, s_ps, AF.Exp,
                                                 scale=SCALE)
                            p_pipe[sp] = p_sb
                        if sp > 0:
                            pv = sp - 1
                            p_sb = p_pipe[pv]
                            for j in range(KPS):
                                kt = KPS * pv + j
                                nc.tensor.matmul(
                                    o_ps,
                                    v_aug[kt][:, h * (HD + 1):
                                              (h + 1) * (HD + 1)],
                                    p_sb[:, j * QWIN:(j + 1) * QWIN],
                                    start=(kt == 0), stop=(kt == NKT - 1))
                        feed(stream)
                        stream += 1
                    # epilogue: snapshot o_ps to SBUF (frees the bank), then
                    # denom row -> DRAM, stride-0 broadcast, divide.
                    o_raw = epi.tile([65, QWIN], F32, tag="oraw")
                    nc.vector.tensor_copy(o_raw, o_ps)
                    nc.sync.dma_start(out=dscr[h][qw, :], in_=o_raw[64:65, :])
                    den_b = epi.tile([64, QWIN], F32, tag="denb")
                    bc_ap = bass.AP(
                        tensor=dscr[h].tensor,
                        offset=dscr[h].offset + qw * QWIN,
                        ap=[[0, 64], [1, QWIN]])
                    nc.sync.dma_start(out=den_b, in_=bc_ap)
                    rden = epi.tile([64, QWIN], F32, tag="rden")
                    nc.vector.reciprocal(rden, den_b)
                    if h % 2 == 0:
                        nc.vector.tensor_mul(
                            ostack[p][0:64, qsl], o_raw[0:64, :], rden)
                    else:
                        # DVE is lane-locked; partitions 64..127 need a DMA
                        # hop to shift the partition base.
                        otmp = epi.tile([64, QWIN], BF16, tag="otmp")
                        nc.vector.tensor_mul(otmp, o_raw[0:64, :], rden)
                        nc.sync.dma_start(
                            out=ostack[p][64:128, qsl], in_=otmp)
                # after the pair's last head, queue its projection
                if h % 2 == 1:
                    for i, (rt, hf) in enumerate(
                            (rt, hf) for rt in range(NKT) for hf in range(2)):
                        push(stream + 2 + i, 512, proj_unit(p, rt, hf))
            # drain remaining dense work (last pair's projection)
            while queue:
                _, _, _, fn = heapq.heappop(queue)
                fn()
    nc.compile()
    return nc


_NC_CACHE = {}


def _get_nc(rep=1):
    if rep not in _NC_CACHE:
        _NC_CACHE[rep] = _build_nc(rep)
    return _NC_CACHE[rep]


def _run(in_maps):
    nc = _get_nc()
    return run_bass_kernel_spmd(nc, in_maps, core_ids=list(range(8)))


def _make_in_maps(x, w_qkv, w_proj):
    import ml_dtypes
    bf16 = ml_dtypes.bfloat16
    x = np.ascontiguousarray(x, dtype=np.float32)
    w_qkv = np.asarray(w_qkv, dtype=np.float32)
    w_proj = np.asarray(w_proj, dtype=np.float32)
    in_maps = []
    for c in range(8):
        b, g = divmod(c, 2)
        wq = w_qkv[:, g * 512:(g + 1) * 512]
        wk = w_qkv[:, D + g * 512:D + (g + 1) * 512]
        wvs = w_qkv[:, 2 * D + g * 512:2 * D + (g + 1) * 512]
        in_maps.append({
            "xt": np.ascontiguousarray(x[b].T).astype(bf16),
            "wqk": np.ascontiguousarray(
                np.concatenate([wq, wk], axis=1)).astype(bf16),
            "wv": np.ascontiguousarray(wvs).astype(bf16),
            "wp": np.ascontiguousarray(w_proj[g * 512:(g + 1) * 512, :]
                                       ).astype(bf16),
        })
    return in_maps


def kernel(x, w_qkv, w_proj, b_proj):
    in_maps = _make_in_maps(x, w_qkv, w_proj)
    res = _run(in_maps)
    out = np.empty((B, N, D), dtype=np.float32)
    bp = np.asarray(b_proj, dtype=np.float32)
    for b in range(B):
        acc = np.zeros((N, D), dtype=np.float32)
        for c in (2 * b, 2 * b + 1):
            ypc = np.asarray(res.results[c]["yp"], dtype=np.float32)
            acc += ypc.reshape(4, N, D).sum(axis=0)
        out[b] = acc + bp
    return out


if __name__ == "__main__":
    rng = np.random.default_rng(0)
    x = rng.standard_normal((B, N, D), dtype=np.float32)
    w_qkv = (rng.standard_normal((D, 3 * D), dtype=np.float32) * D ** -0.5)
    w_proj = (rng.standard_normal((D, D), dtype=np.float32) * D ** -0.5)
    b_proj = np.zeros(D, dtype=np.float32)
    out = kernel(x, w_qkv, w_proj, b_proj)
    print("ran; out shape", out.shape, "mean abs", np.abs(out).mean())


# revision 4
# speedup vs baseline: 1.0638x; 1.0638x over previous
"""Multi-head attention (B=4, N=2048, D=1024, H=16) on 8 trn2 cores, v2.

Sharding: core c -> (batch b = c//2, head-group g = c%2), 8 heads/core.
Host pre-transposes x (-> [D, N]) and casts x/weights to bf16, so the
device runs a pure bf16 matmul pipeline with no PE transposes and no
weight-rounding passes.  Host sums the per-pair projection partials.

Device dataflow per core:
  xp[dt]    : x^T resident in SBUF (DMA, bf16)
  v_aug[kt] : [128 keys, 8*(64+1)] v columns + ones column per head
  qk stage  : per head-pair, q'/k' [128, N] bf16 SBUF-resident; even head
              reads partitions 0..63, odd head 64..127 (PE row tiling —
              fine in bf16; the fp32r base-64 crash does not apply)
  attention : per (pair, 512-q window): 8 strips of 2 key tiles with
              the pair's TWO heads interleaved strip-by-strip (doubles
              the independent work in flight at the same PSUM budget)
              and software-pipelined one strip ahead of the ACT exp;
              QK -> psum [128,1024] (double-buffered), exp (ACT, 1024
              wide) -> p bf16, PV accumulates per-head o_ps [65,512]
              (row 64 = softmax denom via ones column); epilogue:
              denom -> DRAM -> stride-0 bcast, reciprocal, divide
  proj      : per pair, y_part = ostack_p^T @ wp rows -> bf16 partial,
              DMA'd out; host adds the 4 pair partials + b_proj.

A deficit ledger interleaves the dense work (v_aug tiles, next pair's
qk sections, previous pair's proj) into the attention strip stream as
PE filler so the tensor engine never idles waiting on the ACT exp.
"""
import sys

sys.path.insert(0, "/opt/trn_rl_repo")

from collections import deque

import numpy as np

import concourse.bass as bass
import concourse.mybir as mybir
import concourse.tile as tile
from concourse import bacc
from concourse.bass_utils import run_bass_kernel_spmd

F32 = mybir.dt.float32
BF16 = mybir.dt.bfloat16
AF = mybir.ActivationFunctionType

B = 4            # batch
N = 2048         # sequence length
D = 1024         # model dim
H = 16           # total heads
HL = 8           # heads per core
HD = 64          # head dim
SCALE = HD ** -0.5

import os

NKT = N // 128   # 16 key tiles
NDT = D // 128   # 8 d tiles
QWIN = int(os.environ.get("K2_QWIN", "512"))  # q window
NQW = N // QWIN
KPS = int(os.environ.get("K2_KPS", "2"))   # key tiles per strip
SPS_BUFS = int(os.environ.get("K2_SPSB", "2"))  # s_ps double-buffering
HI = int(os.environ.get("K2_HI", "1"))     # interleave the pair's 2 heads
GP_MUL = int(os.environ.get("K2_GPMUL", "0"))  # epilogue divide on GPSIMD
NSP = NKT // KPS # strips per (head, window)
FPH = NQW * (NSP + 1)  # feed slots per head
PAIR_FEEDS = NQW * (NSP + 1) if HI else 2 * FPH

# PE cycles granted to the filler ledger per attention feed slot (the gap
# between ACT exp time and the slot's own QK+PV matmul time).
DEFICIT_C = int(os.environ.get("K2_DEF", "2100" if HI else "1050"))


def _build_nc(rep=1):
    nc = bacc.Bacc(None, target_bir_lowering=False)

    xt = nc.declare_dram_parameter("xt", [D, N], BF16, isOutput=False)
    wqk = nc.declare_dram_parameter("wqk", [D, D], BF16, isOutput=False)
    wv = nc.declare_dram_parameter("wv", [D, 512], BF16, isOutput=False)
    wp = nc.declare_dram_parameter("wp", [512, D], BF16, isOutput=False)
    yp = nc.declare_dram_parameter("yp", [4 * N, D], BF16, isOutput=True)

    with tile.TileContext(nc) as tc:
      with tc.tile_pool(name="dramp", bufs=1, space="DRAM") as dramp:
        dscr = [dramp.tile([NQW, QWIN], F32, tag=f"dscr{h}", name=f"dscr{h}")
                for h in range(HL)]
        for _rep in range(rep):
         with tc.tile_pool(name="wsb", bufs=1) as wsb, \
              tc.tile_pool(name="xr", bufs=1) as xrp, \
              tc.tile_pool(name="resv", bufs=1) as resv, \
              tc.tile_pool(name="qkst", bufs=4) as qkstp, \
              tc.tile_pool(name="ost", bufs=3) as ostp, \
              tc.tile_pool(name="pst", bufs=4 if HI else 3) as pstp, \
              tc.tile_pool(name="epi", bufs=3 if HI else 2) as epi, \
              tc.tile_pool(name="ypst", bufs=2) as ypstp, \
              tc.tile_pool(name="d_ps", bufs=2, space="PSUM") as d_psp, \
              tc.tile_pool(name="s_ps", bufs=SPS_BUFS, space="PSUM") as s_psp, \
              tc.tile_pool(name="o_ps", bufs=int(os.environ.get("K2_OPSB", "2")),
                           space="PSUM") as o_psp:

            # ---- resident loads (DMA only, bf16 straight from DRAM) ----
            xp = [xrp.tile([128, N], BF16, tag=f"xp{dt}", name=f"xp{dt}")
                  for dt in range(NDT)]
            wv_sb = [wsb.tile([128, 512], BF16, tag=f"wv{dt}", name=f"wv{dt}")
                     for dt in range(NDT)]
            wqk_sb = [wsb.tile([128, D], BF16, tag=f"wqk{dt}", name=f"wqk{dt}")
                      for dt in range(NDT)]
            wp_sb = [wsb.tile([128, D], BF16, tag=f"wp{p}", name=f"wp{p}")
                     for p in range(4)]
            for dt in range(NDT):
                nc.sync.dma_start(out=xp[dt],
                                  in_=xt[dt * 128:(dt + 1) * 128, :])
                nc.sync.dma_start(out=wv_sb[dt],
                                  in_=wv[dt * 128:(dt + 1) * 128, :])
            for dt in range(NDT):
                nc.sync.dma_start(out=wqk_sb[dt],
                                  in_=wqk[dt * 128:(dt + 1) * 128, :])
            for p in range(4):
                nc.sync.dma_start(out=wp_sb[p],
                                  in_=wp[p * 128:(p + 1) * 128, :])

            v_aug = [resv.tile([128, HL * (HD + 1)], BF16,
                               tag=f"va{kt}", name=f"va{kt}")
                     for kt in range(NKT)]

            # stage tiles for q'/k', SBUF-resident per pair
            qst = {}
            kst = {}

            # ---------------- dense work units ----------------
            def v_unit(kt):
                def fn():
                    ps = d_psp.tile([128, 512], F32, tag="dps")
                    for dt in range(NDT):
                        nc.tensor.matmul(
                            ps, xp[dt][:, kt * 128:(kt + 1) * 128], wv_sb[dt],
                            start=(dt == 0), stop=(dt == NDT - 1))
                    va3 = v_aug[kt].rearrange("p (h c) -> p h c", h=HL)
                    ps3 = ps.rearrange("p (h c) -> p h c", h=HL)
                    nc.vector.tensor_copy(va3[:, :, 0:HD], ps3)
                    nc.gpsimd.memset(va3[:, :, HD:HD + 1], 1.0)
                return fn

            def sec_unit(p, sec, rw):
                # sec 0 = q cols [p*128:(p+1)*128], sec 1 = k cols [512+...]
                def fn():
                    if rw == 0:
                        st = qkstp.tile([128, N], BF16, tag="qkst",
                                        name=f"st{p}_{sec}_{_rep}")
                        (qst if sec == 0 else kst)[p] = st
                    st = (qst if sec == 0 else kst)[p]
                    col0 = sec * 512 + p * 128
                    ps = d_psp.tile([128, 512], F32, tag="dps")
                    for dt in range(NDT):
                        nc.tensor.matmul(
                            ps, wqk_sb[dt][:, col0:col0 + 128],
                            xp[dt][:, rw * 512:(rw + 1) * 512],
                            start=(dt == 0), stop=(dt == NDT - 1))
                    nc.vector.tensor_copy(st[:, rw * 512:(rw + 1) * 512], ps)
                return fn

            ypst = {}

            def proj_unit(p, rt, half):
                def fn():
                    if half == 0:
                        ypst[p] = ypstp.tile([128, D], BF16, tag="ypst",
                                             name=f"yp{p}_{rt}_{_rep}")
                    ps = d_psp.tile([128, 512], F32, tag="dps")
                    nc.tensor.matmul(
                        ps, ostack[p][:, rt * 128:(rt + 1) * 128],
                        wp_sb[p][:, half * 512:(half + 1) * 512],
                        start=True, stop=True)
                    nc.vector.tensor_copy(
                        ypst[p][:, half * 512:(half + 1) * 512], ps)
                    if half == 1:
                        nc.sync.dma_start(
                            out=yp[p * N + rt * 128:p * N + (rt + 1) * 128, :],
                            in_=ypst[p])
                return fn

            # priority queue of (due_stream_idx, seq, cycles, fn), popped
            # earliest-due first, paced by a PE-cycle credit ledger
            import heapq
            queue = []
            seq_n = [0]

            def push(due, cyc, fn):
                heapq.heappush(queue, (due, seq_n[0], cyc, fn))
                seq_n[0] += 1

            ledger = {"credit": 0}

            def feed(stream):
                ledger["credit"] = min(ledger["credit"] + DEFICIT_C, 8192)
                while queue and (queue[0][0] <= stream
                                 or ledger["credit"] >= queue[0][2]):
                    _, _, cyc, fn = heapq.heappop(queue)
                    fn()
                    ledger["credit"] = max(ledger["credit"] - cyc, -20000)

            # ---------------- upfront phase ----------------
            # minimum preamble: first strip of h0/qw0 needs v_aug[0:KPS],
            # k' rows for key tiles 0..KPS-1 (k sec rw0) and q' window 0.
            for kt in range(KPS):
                v_unit(kt)()
            sec_unit(0, 1, 0)()
            sec_unit(0, 0, 0)()
            # remaining v tiles: strip sp consumes kt KPS*sp..KPS*sp+KPS-1
            for kt in range(KPS, NKT):
                push(max(0, kt // KPS - 1), 4096, v_unit(kt))
            # pair0's remaining sections, due just before their use
            for rw in range(1, 4):
                push(rw - 1, 4096, sec_unit(0, 1, rw))             # k rw
            for rw in range(1, 4):
                # q seq-range rw first used when its q window starts
                due = max(0, (NSP + 1) * (rw * 512 // QWIN) - 3)
                push(due, 4096, sec_unit(0, 0, rw))
            # later pairs' sections: spread across the prior pair's feeds
            for p in range(1, 4):
                base = (p - 1) * PAIR_FEEDS           # start of pair p-1
                for i, sec in enumerate((1, 0)):
                    for rw in range(4):
                        due = base + 4 + 4 * (4 * i + rw)
                        push(due, 4096, sec_unit(p, sec, rw))

            ostack = [None] * 4

            def epilogue(h, qw, o_ps):
                # snapshot o_ps to SBUF (frees the bank), then denom row
                # -> DRAM, stride-0 broadcast, reciprocal, divide.
                p = h // 2
                qsl = slice(qw * QWIN, (qw + 1) * QWIN)
                o_raw = epi.tile([65, QWIN], F32, tag="oraw")
                nc.vector.tensor_copy(o_raw, o_ps)
                nc.sync.dma_start(out=dscr[h][qw, :], in_=o_raw[64:65, :])
                den_b = epi.tile([64, QWIN], F32, tag="denb")
                bc_ap = bass.AP(
                    tensor=dscr[h].tensor,
                    offset=dscr[h].offset + qw * QWIN,
                    ap=[[0, 64], [1, QWIN]])
                nc.sync.dma_start(out=den_b, in_=bc_ap)
                rden = epi.tile([64, QWIN], F32, tag="rden")
                nc.vector.reciprocal(rden, den_b)
                # the divide runs on GPSIMD (SBUF-only operands) to keep
                # DVE free for the PSUM copies
                mul_eng = nc.gpsimd if GP_MUL else nc.vector
                if h % 2 == 0:
                    mul_eng.tensor_mul(
                        ostack[p][0:64, qsl], o_raw[0:64, :], rden)
                else:
                    # engines are lane-locked; partitions 64..127 need a
                    # DMA hop to shift the partition base.
                    otmp = epi.tile([64, QWIN], BF16, tag="otmp")
                    mul_eng.tensor_mul(otmp, o_raw[0:64, :], rden)
                    nc.sync.dma_start(
                        out=ostack[p][64:128, qsl], in_=otmp)

            # ---------------- attention (pair-interleaved) ----------------
            stream = 0
            if HI:
                for pp in range(4):
                    k_t = kst[pp]
                    q_t = qst[pp]
                    ostack[pp] = ostp.tile([128, N], BF16, tag="ostk",
                                           name=f"os{pp}_{_rep}")
                    for qw in range(NQW):
                        qsl = slice(qw * QWIN, (qw + 1) * QWIN)
                        o2 = [o_psp.tile([65, QWIN], F32, tag="ops",
                                         name=f"o{pp}_{qw}_{hh}_{_rep}")
                              for hh in range(2)]
                        pp2 = [[None] * NSP, [None] * NSP]
                        for sp in range(NSP + 1):
                            if sp < NSP:
                                for hh in range(2):
                                    half = slice(hh * 64, hh * 64 + 64)
                                    s_ps = s_psp.tile([128, KPS * QWIN], F32,
                                                      tag="sps")
                                    for j in range(KPS):
                                        kt = KPS * sp + j
                                        nc.tensor.matmul(
                                            s_ps[:, j * QWIN:(j + 1) * QWIN],
                                            k_t[half, kt * 128:(kt + 1) * 128],
                                            q_t[half, qsl],
                                            start=True, stop=True)
                                    p_sb = pstp.tile([128, KPS * QWIN], BF16,
                                                     tag="psb")
                                    nc.scalar.activation(p_sb, s_ps, AF.Exp,
                                                         scale=SCALE)
                                    pp2[hh][sp] = p_sb
                            if sp > 0:
                                for hh in range(2):
                                    h = 2 * pp + hh
                                    p_sb = pp2[hh][sp - 1]
                                    for j in range(KPS):
                                        kt = KPS * (sp - 1) + j
                                        nc.tensor.matmul(
                                            o2[hh],
                                            v_aug[kt][:, h * (HD + 1):
                                                      (h + 1) * (HD + 1)],
                                            p_sb[:, j * QWIN:(j + 1) * QWIN],
                                            start=(kt == 0),
                                            stop=(kt == NKT - 1))
                            feed(stream)
                            stream += 1
                        for hh in range(2):
                            epilogue(2 * pp + hh, qw, o2[hh])
                    for i, (rt, hf) in enumerate(
                            (rt, hf) for rt in range(NKT) for hf in range(2)):
                        push(stream + 2 + i, 512, proj_unit(pp, rt, hf))
            for h in (range(0) if HI else range(HL)):
                p = h // 2
                half = slice((h % 2) * 64, (h % 2) * 64 + 64)
                k_t = kst[p]
                q_t = qst[p]
                if h % 2 == 0:
                    ostack[p] = ostp.tile([128, N], BF16, tag="ostk",
                                          name=f"os{p}_{_rep}")
                for qw in range(NQW):
                    qsl = slice(qw * QWIN, (qw + 1) * QWIN)
                    o_ps = o_psp.tile([65, QWIN], F32, tag="ops")
                    # software pipeline: QK(i)+exp(i) emitted a strip ahead
                    # of PV(i), so the in-order PE never waits on the ACT.
                    p_pipe = [None] * NSP
                    for sp in range(NSP + 1):
                        if sp < NSP:
                            s_ps = s_psp.tile([128, KPS * QWIN], F32,
                                              tag="sps")
                            for j in range(KPS):
                                kt = KPS * sp + j
                                nc.tensor.matmul(
                                    s_ps[:, j * QWIN:(j + 1) * QWIN],
                                    k_t[half, kt * 128:(kt + 1) * 128],
                                    q_t[half, qsl], start=True, stop=True)
                            p_sb = pstp.tile([128, KPS * QWIN], BF16,
                                             tag="psb")
                            nc.scalar.activation(p_sb, s_ps, AF.Exp,
                                                 scale=SCALE)
                            p_pipe[sp] = p_sb
                        if sp > 0:
                            pv = sp - 1
                            p_sb = p_pipe[pv]
                            for j in range(KPS):
                                kt = KPS * pv + j
                                nc.tensor.matmul(
                                    o_ps,
                                    v_aug[kt][:, h * (HD + 1):
                                              (h + 1) * (HD + 1)],
                                    p_sb[:, j * QWIN:(j + 1) * QWIN],
                                    start=(kt == 0), stop=(kt == NKT - 1))
                        feed(stream)
                        stream += 1
                    epilogue(h, qw, o_ps)
                # after the pair's last head, queue its projection
                if h % 2 == 1:
                    for i, (rt, hf) in enumerate(
                            (rt, hf) for rt in range(NKT) for hf in range(2)):
                        push(stream + 2 + i, 512, proj_unit(p, rt, hf))
            # drain remaining dense work (last pair's projection)
            while queue:
                _, _, _, fn = heapq.heappop(queue)
                fn()
    nc.compile()
    return nc


_NC_CACHE = {}


def _get_nc(rep=1):
    if rep not in _NC_CACHE:
        _NC_CACHE[rep] = _build_nc(rep)
    return _NC_CACHE[rep]


def _run(in_maps):
    nc = _get_nc()
    return run_bass_kernel_spmd(nc, in_maps, core_ids=list(range(8)))


def _make_in_maps(x, w_qkv, w_proj):
    import ml_dtypes
    bf16 = ml_dtypes.bfloat16
    x = np.ascontiguousarray(x, dtype=np.float32)
    w_qkv = np.asarray(w_qkv, dtype=np.float32)
    w_proj = np.asarray(w_proj, dtype=np.float32)
    in_maps = []
    for c in range(8):
        b, g = divmod(c, 2)
        wq = w_qkv[:, g * 512:(g + 1) * 512]
        wk = w_qkv[:, D + g * 512:D + (g + 1) * 512]
        wvs = w_qkv[:, 2 * D + g * 512:2 * D + (g + 1) * 512]
        in_maps.append({
            "xt": np.ascontiguousarray(x[b].T).astype(bf16),
            "wqk": np.ascontiguousarray(
                np.concatenate([wq, wk], axis=1)).astype(bf16),
            "wv": np.ascontiguousarray(wvs).astype(bf16),
            "wp": np.ascontiguousarray(w_proj[g * 512:(g + 1) * 512, :]
                                       ).astype(bf16),
        })
    return in_maps


def kernel(x, w_qkv, w_proj, b_proj):
    in_maps = _make_in_maps(x, w_qkv, w_proj)
    res = _run(in_maps)
    out = np.empty((B, N, D), dtype=np.float32)
    bp = np.asarray(b_proj, dtype=np.float32)
    for b in range(B):
        acc = np.zeros((N, D), dtype=np.float32)
        for c in (2 * b, 2 * b + 1):
            ypc = np.asarray(res.results[c]["yp"], dtype=np.float32)
            acc += ypc.reshape(4, N, D).sum(axis=0)
        out[b] = acc + bp
    return out


if __name__ == "__main__":
    rng = np.random.default_rng(0)
    x = rng.standard_normal((B, N, D), dtype=np.float32)
    w_qkv = (rng.standard_normal((D, 3 * D), dtype=np.float32) * D ** -0.5)
    w_proj = (rng.standard_normal((D, D), dtype=np.float32) * D ** -0.5)
    b_proj = np.zeros(D, dtype=np.float32)
    out = kernel(x, w_qkv, w_proj, b_proj)
    print("ran; out shape", out.shape, "mean abs", np.abs(out).mean())
